# revision 18
# baseline (speedup 1.0000x reference)
"""Trainium2 Bass kernel for nn_CausalDerivative (per-node MLP stack).

Computation (reference):
    x = where(discrete_mask, (inputs > 0), inputs)          # straight-through gate
    W1m = W1 * M[:, None, :]   (M = adjacency, last row one-hot @ last col)
    h = relu(einsum('bn,ihn->bih', x, W1m))                 # [B, N, H]
    out = einsum('bih,ih->bi', h, W2)                       # [B, N]

Strategy: pure data-parallel over 8 NeuronCores (batch sharded 4096/core).
The gate and all weight folding happen on the host. On each core:

  stage 1:  z[ih, b] = W1slot[n, ih]^T @ x^T[n, b]   as 2 concurrent
            row-tile matmuls (K=64, M=128) per 128-unit slot, f32 into
            3x [128, 1024] PSUM tiles (2 banks each, triple-buffered)
  evict:    relu PSUM -> SBUF bf16, one [128, 1024] op per PSUM tile,
            greedily load-balanced between ACT and DVE (the bottleneck:
            both engines read PSUM at ~1 elem/lane/cycle, so the kernel
            floor is the 16.8M-element PSUM drain at ~583 ns/slot)
  stage 2:  out[node, b] += w2blk^T @ relu_z   as 4 concurrent column-
            strip matmuls (K=128, M=32) per slot pair, PSUM-accumulating
            16 slot pairs directly into one [128, 512] out bank
            (node-major: no cross-accumulator adds needed at the end)

Slots are chunk pairs (q, q+16) interleaved so each stage-2 group covers
nodes 2q,2q+1 (strip partitions 0:64 batchA / 64:128 batchB). All 128
fills of the 4 batch pairs form one continuous stream (stage-2 groups,
out-bank eviction and output DMA of a pair overlap the next pair's
fills), keeping both PSUM-reading engines >98% busy end to end.
"""

import os
import sys
import numpy as np


def _ensure_axon_hooks():
    """The NTFF trace path imports antenv.axon_hooks; provide it if absent."""
    try:
        import antenv.axon_hooks  # noqa: F401
        return
    except ImportError:
        pass
    try:
        import types
        import antenv

        mod = types.ModuleType("antenv.axon_hooks")
        mod._NTFF_PROFILE_HOOK = None

        def set_axon_ntff_profile_hook(hook):
            mod._NTFF_PROFILE_HOOK = hook

        def get_axon_ntff_profile_hook():
            return mod._NTFF_PROFILE_HOOK

        mod.set_axon_ntff_profile_hook = set_axon_ntff_profile_hook
        mod.get_axon_ntff_profile_hook = get_axon_ntff_profile_hook
        sys.modules["antenv.axon_hooks"] = mod
        antenv.axon_hooks = mod
        try:
            from trn_agent_boot.trn_boot import _ntff_profile_via_ctypes

            hook = _ntff_profile_via_ctypes("/opt/axon/libaxon_pjrt.so")
            if hook is not None:
                mod._NTFF_PROFILE_HOOK = hook
        except Exception:
            pass
    except Exception:
        pass


_ensure_axon_hooks()

import concourse.bass as bass
import concourse.tile as tile
from concourse import mybir, bacc
from concourse.bass_utils import run_bass_kernel_spmd

B, N, H = 32768, 64, 64
IH = N * H                    # 4096 hidden units total
N_CORES = 8
BL = B // N_CORES             # 4096 batch rows per core
HALF = BL // 2                # 2048 (batch half per SBUF partition group)
BW = 512                      # batch tile width (PE moving free dim)
NPAIR = HALF // BW            # 4 batch pairs per core
NSLOT = 32                    # 128-unit slots per pair (= chunks)
NPIECE = 2 * NSLOT            # 512-col z pieces per pair (A+B per slot)
FILL = 1024                   # eviction tile width (2 PSUM banks, = 1 slot)
NFILL = (NPIECE * BW + FILL - 1) // FILL   # 32 fills per pair (1 slot each)

F32 = mybir.dt.float32
BF16 = mybir.dt.bfloat16
DT = BF16
import ml_dtypes
NP_DT = ml_dtypes.bfloat16

# exec time of the last traced run (ns), for the test harness
LAST_EXEC_NS = None

_compiled = {}

# cost model (ns) used for greedy ACT/DVE eviction balancing; constants
# calibrated against measured ACTIVATE/TENSOR_SCALAR durations on HW
def _act_cost(fd):
    return 115 + (172 + fd) / 1.2


def _dve_cost(fd):
    return 32 + (120 + fd) / 0.96


def _build_module():
    nc = bacc.Bacc("TRN2", target_bir_lowering=False, debug=False)
    xs = nc.dram_tensor("xs", [128, HALF], DT, kind="ExternalInput").ap()
    w1 = nc.dram_tensor("w1", [128, IH], DT, kind="ExternalInput").ap()
    w2 = nc.dram_tensor("w2", [128, NSLOT * 32], DT, kind="ExternalInput").ap()
    out = nc.dram_tensor("out", [128, HALF], F32, kind="ExternalOutput").ap()

    ds = bass.ds
    Relu = mybir.ActivationFunctionType.Relu

    with tile.TileContext(nc) as tc:
        with (
            tc.tile_pool(name="consts", bufs=1) as consts,
            tc.tile_pool(name="hb", bufs=1) as hbp,
            tc.tile_pool(name="so", bufs=2) as sop,
            tc.tile_pool(name="zt", bufs=3, space="PSUM") as zpool,
            tc.tile_pool(name="ob", bufs=2, space="PSUM") as opool,
        ):
            sx = consts.tile([128, HALF], DT)
            w1s = consts.tile([128, IH], DT)
            w2s = consts.tile([128, NSLOT * 32], DT)
            dummy = consts.tile([128, 1024], DT)
            hbuf = hbp.tile([128, NPIECE * BW], DT)   # 32 KB/partition ring

            # ---- startup DMAs (HWDGE only), critical slices first ----
            nc.sync.dma_start(sx[:, 0:256], xs[:, 0:256])
            nc.scalar.dma_start(sx[:, 256:BW], xs[:, 256:BW])
            nc.sync.dma_start(w1s[:, 0:512], w1[:, 0:512])
            nc.scalar.dma_start(w1s[:, 512:2048], w1[:, 512:2048])
            nc.sync.dma_start(w2s[:, :], w2[:, :])
            nc.scalar.dma_start(w1s[:, 2048:IH], w1[:, 2048:IH])
            nc.sync.dma_start(sx[:, BW:HALF], xs[:, BW:HALF])

            # ---- PE warmup during the DMA wait (HAM un-throttle) ----
            nc.vector.memset(dummy[:], 0.0)
            wtile = zpool.tile([128, FILL], F32, tag="z")
            for _ in range(8):
                nc.tensor.matmul(
                    wtile[0:64, 0:256], dummy[0:64, 0:64], dummy[0:64, 0:256],
                    start=True, stop=True,
                )

            # greedy engine-balancing state for PSUM-read ops
            eng_load = {"act": 0.0, "dve": 0.0}

            def evict(dst, src, fd):
                ca, cd = _act_cost(fd), _dve_cost(fd)
                if eng_load["act"] + ca <= eng_load["dve"] + cd:
                    eng_load["act"] += ca
                    nc.scalar.activation(dst, src, Relu)
                else:
                    eng_load["dve"] += cd
                    nc.vector.tensor_scalar_max(dst, src, 0.0)

            def copy_out(dst, src, fd):
                ca, cd = _act_cost(fd), _dve_cost(fd)
                if eng_load["act"] + ca <= eng_load["dve"] + cd:
                    eng_load["act"] += ca
                    nc.scalar.copy(dst, src)
                else:
                    eng_load["dve"] += cd
                    nc.vector.tensor_copy(dst, src)

            def stage2(q, ob):
                st, sp = q == 0, q == NSLOT // 2 - 1
                for strip in range(4):
                    # strips: 0=nodesLo/A 1=nodesHi/A 2=nodesLo/B 3=nodesHi/B
                    slot = 2 * q + (strip & 1)
                    col = 2048 * q + 1024 * (strip & 1) + 512 * (strip >> 1)
                    nc.tensor.matmul(
                        ob[ds(32 * strip, 32), :],
                        w2s[:, ds(32 * slot, 32)],
                        hbuf[:, ds(col, BW)],
                        start=st,
                        stop=sp,
                        skip_group_check=True,
                        tile_position=(0, 32 * strip),
                    )

            # one continuous fill stream across all 4 batch pairs; stage-2
            # groups are emitted DELAY fills after the fill holding their
            # last column, so the in-order PE queue never head-blocks and
            # the evictors see no pair-boundary bubble.
            DELAY = 3
            PPF = FILL // BW      # z pieces per fill
            sched = {}
            for p in range(NPAIR):
                for q in range(NSLOT // 2):
                    g_ready = NFILL * p + (2048 * q + 2047) // FILL
                    sched.setdefault(g_ready + DELAY, []).append((p, q))
            obs = {}
            for g in range(NFILL * NPAIR + DELAY + 1):
                p, f = divmod(g, NFILL)
                if p < NPAIR:
                    if f == 0:
                        obs[p] = opool.tile([128, BW], F32, tag="ob", name="ob")
                    bs = ds(p * BW, BW)
                    k0 = PPF * f
                    npc = min(PPF, NPIECE - k0)
                    used = npc * BW
                    z = zpool.tile([128, FILL], F32, tag="z")
                    for j in range(npc):
                        k = k0 + j
                        t, half = k >> 1, k & 1
                        r0 = 64 * half
                        nc.tensor.matmul(
                            z[:, ds(j * BW, BW)],
                            w1s[ds(r0, 64), ds(128 * t, 128)],
                            sx[ds(r0, 64), bs],
                            start=True, stop=True,
                        )
                    evict(hbuf[:, ds(f * FILL, used)], z[:, 0:used], used)
                for pp, q in sched.pop(g, ()):
                    stage2(q, obs[pp])
                    if q == NSLOT // 2 - 1:
                        # ---- out bank -> SBUF -> DRAM ----
                        osb = sop.tile([128, BW], F32, tag="osb")
                        copy_out(osb[:], obs.pop(pp)[:], BW)
                        nc.sync.dma_start(out[:, ds(pp * BW, BW)], osb[:])

    nc.compile()
    return nc


def kernel(t, inputs, W1, W2, adjacency, discrete_mask, **_ignored):
    global LAST_EXEC_NS
    inputs = np.asarray(inputs, np.float32)
    W1 = np.asarray(W1, np.float32)
    W2 = np.asarray(W2, np.float32)
    adjacency = np.asarray(adjacency, np.float32)
    discrete_mask = np.asarray(discrete_mask).astype(bool)

    # ---- host-side straight-through gate ----
    x = np.where(discrete_mask[None, :], (inputs > 0).astype(np.float32), inputs)

    # ---- host-side weight folding / layout ----
    M = adjacency.copy()
    one_hot_last = np.zeros(N, np.float32)
    one_hot_last[-1] = 1.0
    M[-1] = M[-1] * one_hot_last
    W1m = W1 * M[:, None, :]                            # [N, H, N]
    W1cols = np.ascontiguousarray(W1m.reshape(IH, N).T)  # [n, ih]

    # slot order: slot 2q -> chunk q (nodes 2q,2q+1), slot 2q+1 -> chunk 16+q
    slot_chunk = np.empty(NSLOT, np.int64)
    slot_chunk[0::2] = np.arange(16)
    slot_chunk[1::2] = np.arange(16) + 16

    w1slots = np.empty((64, IH), np.float32)
    w2d = np.zeros((128, NSLOT * 32), np.float32)
    for tslot, c in enumerate(slot_chunk):
        w1slots[:, 128 * tslot:128 * tslot + 128] = W1cols[:, 128 * c:128 * c + 128]
        # stationary [128, 32] for this slot: rows = units of nodes 2c,2c+1
        p = np.arange(128)
        node = 2 * c + (p >= 64)
        hh = p % 64
        mcol = node - 32 * (c >= 16)
        w2d[p, 32 * tslot + mcol] = W2[node, hh]
    w1d = np.concatenate([w1slots, w1slots], axis=0)     # [128, IH]

    xT = np.ascontiguousarray(x.T)                       # [N, B]

    if 0 not in _compiled:
        _compiled[0] = _build_module()
    nc = _compiled[0]

    w1d_d = w1d.astype(NP_DT)
    w2d_d = w2d.astype(NP_DT)
    xT_d = xT.astype(NP_DT)
    in_maps = []
    for c in range(N_CORES):
        base = c * BL
        xs_c = np.concatenate(
            [xT_d[:, base:base + HALF], xT_d[:, base + HALF:base + BL]], axis=0
        )
        in_maps.append({
            "xs": np.ascontiguousarray(xs_c),
            "w1": w1d_d,
            "w2": w2d_d,
        })

    trace = bool(int(os.environ.get("KERNEL_TRACE", "0")))
    res = run_bass_kernel_spmd(
        nc, in_maps, core_ids=list(range(N_CORES)), trace=trace
    )
    if trace:
        LAST_EXEC_NS = res.exec_time_ns

    outT = np.empty((N, B), np.float32)
    for c in range(N_CORES):
        o = res.results[c]["out"]                        # [128, 2048]
        base = c * BL
        outT[:, base:base + HALF] = o[0:64, :]
        outT[:, base + HALF:base + BL] = o[64:128, :]
    return np.ascontiguousarray(outT.T)
